# revision 52
# baseline (speedup 1.0000x reference)
"""Trainium2 Bass kernel for nn_AttentionHead (sparse attention, 8 cores).

Reference computation (per batch b):
    q = x_q @ wq^T ; k = x_k @ wk^T ; v = x_v @ wv^T          # [S, H]
    s = (q @ k^T) / sqrt(H)                                    # [S, S]
    s = where(mask == 0, 0, s)       # multiplicative 0/1 mask BEFORE softmax
    p = softmax(s, axis=-1)          # masked entries contribute exp(0)=1
    out = p @ v                                                # [S, H]

Sharding: 8 cores; core c -> batch c//2.  TWO launches:
  L1: core c projects k and v for key half c%2 only (removing the k/v
      projection duplication between the two cores of a batch) and ships
      kT = fp8(k/4) plus the compensated value pair v8 = fp8(v),
      vr8 = fp8(v - v8) back to DRAM; the host concatenates the halves.
  L2: core c computes attention for query rows (c%2)*2048 ... +2048
      against all 4096 keys.

All heavy matmuls run fp8 DoubleRow (0.5 cyc/row, K=256/instruction)
with error compensation to hold the 2e-2 gate:
  scores = k4s8 @ (q48 + qr48): k4s8 = fp8(k/4) (single shared tile),
    q48 = fp8(4q) (wq host-scaled by 4 so the PSUM is 4q), qr48 =
    fp8(4q - q48).  The residual removes the q-side fp8 error; the
    k-side ~2.9% remains -> ~1.3e-2 output contribution.
  P = m*(E-1) + 1 identity: the chip computes raw = sum m*(E-1)*v_ext;
    fp8((E-1)*m) has 2.4x less quantization error than fp8(E*m) since
    rms(E-1) = 0.82 vs rms(E) = 1.44.  Host finish:
    out = (raw[:, :H] + colsum(v_eff)) / (raw[:, H] + S).
  PV = pt'8^T @ (v8 + vr8): two fp8-DR matmuls into one accumulator.
  pt' path: ACT exp (f32 PSUM -> bf16), DVE tensor_scalar_sub (E-1, 2x
    rate), Pool tensor_tensor multiply with the fp8 mask -> fp8
    (scalar_tensor_tensor is not ISA-legal on Pool; this split is).
  q-projection runs compensated fp8-DR: x_q = x8 + xr8 (two fp8
    tensors, same bytes as bf16), wq (host-scaled 64x) = w8 + wr8;
    psum = x8@w8 + xr8@w8 + x8@wr8 = 64q in three fp8-DR streams at
    75% of the bf16 matmul cost, and ~2x LESS input-quantization error
    than bf16 x/w (residuals bound the error at ~0.1%).  The xr8@wr8
    term is dropped (~0.03% of q, below quantization noise).
  k/v projections stay bf16 (L1 is DMA-bound; fp8-DR would not help).

Pipeline notes: L2 keeps a 1-buf "warm" PSUM score pool alive until
the PV accumulators take its banks at gt = LOOKP, giving the exp
stream a third score buffer across the q-projection interruptions.
LOOKP = 16 (a full block of pts lag): each +2 of lag removed ~0.9us of
per-block PV/score boundary coupling; the final block's PV runs
entirely in a j2-major drain whose output copies overlap the chains.

CoreSim cost-model: L1 39.3us + L2 88.4us = 127.7us/core (baseline
149.2us).  HW-validated rel err 1.812e-2 (gate 2e-2), deterministic.
"""

import numpy as np
import ml_dtypes

import concourse.bass as bass
import concourse.mybir as mybir
import concourse.tile as tile
from concourse import bacc
from concourse.bass_utils import run_bass_kernel_spmd

F32 = mybir.dt.float32
BF16 = mybir.dt.bfloat16
FP8 = mybir.dt.float8e4

B, S, DV, H = 4, 4096, 1024, 256
N_CORES = 8
CORES_PER_BATCH = N_CORES // B
SQL = S // CORES_PER_BATCH
SKH = S // 2                  # keys per core in L1

QSC = 4.0


def build_kv_nc(SKH_, DV_, H_, num_devices=1):
    """L1: kTh = fp8(k/4) [H, SKH]; v8h/vr8h [SKH/2, 2, H+1] fp8
    (row r = pr*128+p, slot u -> sk = pr*256 + u*128 + p)."""
    P = 128
    SKB = 512
    DC = DV_ // P
    NSKB = SKH_ // SKB
    HC = H_ // P
    NPR = SKH_ // 256             # v pair tiles

    nc = bacc.Bacc("TRN2", target_bir_lowering=False, debug=False,
                   num_devices=num_devices)

    x_kT = nc.dram_tensor("x_kTh", [DV_, SKH_], BF16, kind="ExternalInput").ap()
    x_vT = nc.dram_tensor("x_vTh", [DV_, SKH_], BF16, kind="ExternalInput").ap()
    wkT = nc.dram_tensor("wkT", [DV_, H_], BF16, kind="ExternalInput").ap()
    wvT = nc.dram_tensor("wvT", [DV_, H_], BF16, kind="ExternalInput").ap()
    kTh = nc.dram_tensor("kTh", [H_, SKH_], FP8, kind="ExternalOutput").ap()
    # v8 and vr8 interleaved in one tensor: vv8h[r, u, 0, :] = v8,
    # vv8h[r, u, 1, :] = vr8 -> one DMA per pair, 514+B descriptors
    vv8h = nc.dram_tensor("vv8h", [NPR * P, 2, 2, H_ + 1], FP8,
                          kind="ExternalOutput").ap()

    with tile.TileContext(nc) as tc:
        with (
            tc.tile_pool(name="weights", bufs=2) as w_pool,
            tc.tile_pool(name="xT", bufs=3) as xT_pool,
            tc.tile_pool(name="kt", bufs=3) as kt_pool,
            tc.tile_pool(name="v8", bufs=4) as v8_pool,
            tc.tile_pool(name="kps", bufs=2, space="PSUM") as k_ps,
            tc.tile_pool(name="vps", bufs=2, space="PSUM") as v_ps,
        ):
            w_sb = {}

            def load_w(name, wT, split=False):
                t = w_pool.tile([P, DC, H_], BF16, tag=f"w_{name}",
                                name=f"w_{name}")
                src_ap = wT.rearrange("(dc p) h -> p dc h", p=P)
                if split:
                    nc.sync.dma_start(out=t[:, 0:1, :], in_=src_ap[:, 0:1, :])
                    nc.sync.dma_start(out=t[:, 1:DC, :], in_=src_ap[:, 1:DC, :])
                else:
                    nc.sync.dma_start(out=t[:], in_=src_ap)
                w_sb[name] = t

            load_w("k", wkT, split=True)

            xv_sb = [None] * NSKB

            def issue_xv(skb):
                if skb >= NSKB or xv_sb[skb] is not None:
                    return
                t = xT_pool.tile([P, DC, SKB], BF16, tag="xvT",
                                 name=f"xv_{skb}")
                nc.sync.dma_start(
                    out=t[:],
                    in_=x_vT[:, skb * SKB:(skb + 1) * SKB].rearrange(
                        "(dc p) n -> p dc n", p=P))
                xv_sb[skb] = t

            def k_block(skb):
                xk_t = xT_pool.tile([P, DC, SKB], BF16, tag="xkT",
                                    name=f"xk_{skb}")
                xk_src = x_kT[:, skb * SKB:(skb + 1) * SKB].rearrange(
                    "(dc p) n -> p dc n", p=P)
                if skb == 0:
                    for a, b in ((0, 1), (1, 2), (2, 4), (4, DC)):
                        nc.sync.dma_start(out=xk_t[:, a:b, :],
                                          in_=xk_src[:, a:b, :])
                else:
                    nc.sync.dma_start(out=xk_t[:], in_=xk_src)
                kt = kt_pool.tile([P, HC, SKB], FP8, tag="kT",
                                  name=f"kT_{skb}")
                for hc in range(HC):
                    ps = k_ps.tile([P, SKB], F32, tag="kps")
                    for dc in range(DC):
                        nc.tensor.matmul(
                            ps[:],
                            w_sb["k"][:, dc, hc * P:(hc + 1) * P],
                            xk_t[:, dc, :],
                            start=(dc == 0), stop=(dc == DC - 1))
                    nc.vector.tensor_scalar_mul(kt[:, hc, :], ps[:],
                                                1.0 / QSC)
                nc.scalar.dma_start(
                    out=kTh[:, skb * SKB:(skb + 1) * SKB].rearrange(
                        "(hc p) n -> p hc n", p=P),
                    in_=kt[:])

            def v_pair(pr):
                vvt = v8_pool.tile([P, 2, 2, H_ + 1], FP8, tag="vv8",
                                   name=f"vv8_{pr}")
                for u in (0, 1):
                    kc = 2 * pr + u
                    skb, j = divmod(kc, SKB // P)
                    ps = v_ps.tile([P, H_], F32, tag="vps")
                    for dc in range(DC):
                        nc.tensor.matmul(
                            ps[:],
                            xv_sb[skb][:, dc, j * P:(j + 1) * P],
                            w_sb["v"][:, dc, :],
                            start=(dc == 0), stop=(dc == DC - 1))
                    nc.vector.tensor_copy(vvt[:, u, 0, 0:H_], ps[:])
                    nc.vector.scalar_tensor_tensor(
                        vvt[:, u, 1, 0:H_], ps[:], 1.0, vvt[:, u, 0, 0:H_],
                        op0=mybir.AluOpType.mult,
                        op1=mybir.AluOpType.subtract)
                nc.gpsimd.memset(vvt[:, :, 0, H_:H_ + 1], 1.0)
                nc.gpsimd.memset(vvt[:, :, 1, H_:H_ + 1], 0.0)
                # out-DMA issued from the (otherwise idle) ACT queue so
                # the SP queue only carries the input stream
                nc.scalar.dma_start(out=vv8h[pr * P:(pr + 1) * P, :, :, :],
                                    in_=vvt[:])

            # interleave k and v blocks: they are independent, so the
            # PE alternates while the bus streams xk/xv back to back
            load_w("v", wvT)
            for skb in range(NSKB):
                k_block(skb)
                issue_xv(skb)
                v_pair(2 * skb)
                v_pair(2 * skb + 1)

    nc.compile()
    return nc


def build_attn_nc(SQL_, SK_, DV_, H_, scale, num_devices=1):
    """L2: q-projection + attention; kT/v8/vr8 come from DRAM (L1)."""
    P = 128
    SKB = 512
    DC = DV_ // P
    NSKB = SK_ // SKB
    NKC = SK_ // P
    NSQB = SQL_ // SKB
    HC = H_ // P
    NPAIR = NKC // 2
    NPRT = SK_ // 256             # v pair tiles total
    LOOKP = 16

    nc = bacc.Bacc("TRN2", target_bir_lowering=False, debug=False,
                   num_devices=num_devices)

    xq8T = nc.dram_tensor("xq8T", [DV_, SQL_], FP8, kind="ExternalInput").ap()
    xqr8T = nc.dram_tensor("xqr8T", [DV_, SQL_], FP8,
                           kind="ExternalInput").ap()
    kT_in = nc.dram_tensor("kT_in", [H_, SK_], FP8, kind="ExternalInput").ap()
    vv8_in = nc.dram_tensor("vv8_in", [NPRT * P, 2, 2, H_ + 1], FP8,
                            kind="ExternalInput").ap()
    maskT = nc.dram_tensor("maskT", [SK_, SQL_], FP8, kind="ExternalInput").ap()
    wq8T = nc.dram_tensor("wq8T", [DV_, H_], FP8, kind="ExternalInput").ap()
    wqr8T = nc.dram_tensor("wqr8T", [DV_, H_], FP8, kind="ExternalInput").ap()
    out = nc.dram_tensor("out", [SQL_, H_ + 1], F32, kind="ExternalOutput").ap()

    with tile.TileContext(nc) as tc:
        with (
            tc.tile_pool(name="weights", bufs=1) as w_pool,
            tc.tile_pool(name="qT", bufs=NSQB) as qT_pool,
            tc.tile_pool(name="qrT", bufs=NSQB) as qrT_pool,
            tc.tile_pool(name="kT", bufs=NSKB) as kT_pool,
            tc.tile_pool(name="v8", bufs=NPRT) as v8_pool,
            tc.tile_pool(name="maskp", bufs=7) as mask_pool,
            tc.tile_pool(name="xq", bufs=4) as xq_pool,
        ):
            # q weights first, then xq0 — these gate q_proj(0) and thus
            # the whole B pipeline; kT tiles stream in behind them.
            # Compensated-fp8 q path: psum accumulates
            # (x8+xr8)@w8 + x8@wr8 = 64q (w host-scaled by 64), three
            # fp8-DR streams at 75% of the bf16 projection cost.
            wq8_sb = w_pool.tile([P, DC, H_], FP8, tag="w_q8", name="w_q8")
            nc.sync.dma_start(out=wq8_sb[:],
                              in_=wq8T.rearrange("(dc p) h -> p dc h", p=P))
            wqr8_sb = w_pool.tile([P, DC, H_], FP8, tag="w_qr8",
                                  name="w_qr8")
            nc.sync.dma_start(out=wqr8_sb[:],
                              in_=wqr8T.rearrange("(dc p) h -> p dc h", p=P))

            kT_sb = [None] * NSKB

            def issue_kT(skb):
                if skb >= NSKB or kT_sb[skb] is not None:
                    return
                t = kT_pool.tile([P, HC, SKB], FP8, tag="kT",
                                 name=f"kT_{skb}")
                nc.sync.dma_start(
                    out=t[:],
                    in_=kT_in[:, skb * SKB:(skb + 1) * SKB].rearrange(
                        "(hc p) n -> p hc n", p=P))
                kT_sb[skb] = t

            vv8_sb = [None] * NPRT

            def issue_v(pr):
                if pr >= NPRT or vv8_sb[pr] is not None:
                    return
                t = v8_pool.tile([P, 2, 2, H_ + 1], FP8, tag="vv8",
                                 name=f"vv8_{pr}")
                nc.sync.dma_start(out=t[:],
                                  in_=vv8_in[pr * P:(pr + 1) * P, :, :, :])
                vv8_sb[pr] = t

            xq_sb = [None] * NSQB
            xqr_sb = [None] * NSQB

            def issue_xq(sqb):
                if sqb >= NSQB or xq_sb[sqb] is not None:
                    return
                t = xq_pool.tile([P, DC, SKB], FP8, tag="xq",
                                 name=f"xq_{sqb}")
                src = xq8T[:, sqb * SKB:(sqb + 1) * SKB].rearrange(
                    "(dc p) n -> p dc n", p=P)
                tr = xq_pool.tile([P, DC, SKB], FP8, tag="xqr",
                                  name=f"xqr_{sqb}")
                srcr = xqr8T[:, sqb * SKB:(sqb + 1) * SKB].rearrange(
                    "(dc p) n -> p dc n", p=P)
                if sqb == 0:
                    for a, b in ((0, 2), (2, 4), (4, DC)):
                        nc.sync.dma_start(out=t[:, a:b, :], in_=src[:, a:b, :])
                else:
                    nc.sync.dma_start(out=t[:], in_=src)
                nc.sync.dma_start(out=tr[:], in_=srcr)
                xq_sb[sqb] = t
                xqr_sb[sqb] = tr

            MG = 8
            NMG = NKC // MG
            m_chunks = {}

            def issue_mask_chunk(sqb, g, split=False):
                if (sqb, g) in m_chunks or sqb >= NSQB:
                    return
                t = mask_pool.tile([P, MG, SKB], FP8, tag="maskT",
                                   name=f"mask_{sqb}_{g}")
                src_ap = maskT[g * MG * P:(g + 1) * MG * P,
                               sqb * SKB:(sqb + 1) * SKB].rearrange(
                                   "(kc p) n -> p kc n", p=P)
                if split:
                    nc.sync.dma_start(out=t[:, 0:2, :], in_=src_ap[:, 0:2, :])
                    nc.sync.dma_start(out=t[:, 2:MG, :], in_=src_ap[:, 2:MG, :])
                else:
                    nc.sync.dma_start(out=t[:], in_=src_ap)
                m_chunks[(sqb, g)] = t

            qT_sb = [None] * NSQB
            qrT_sb = [None] * NSQB

            NB = NSQB * NPAIR
            o_ps_blk = {}
            pts = {}
            o_psum_pool = None
            with (
                tc.tile_pool(name="ep", bufs=4) as e_pool,
                tc.tile_pool(name="e1p", bufs=4) as e1_pool,
                tc.tile_pool(name="ptp", bufs=LOOKP + 3) as pt_pool,
                tc.tile_pool(name="osb", bufs=6) as o_sb_pool,
                tc.tile_pool(name="s2psum", bufs=2, space="PSUM") as s2_pool,
            ):
                proj_ps = tc.alloc_tile_pool(name="projps", bufs=2,
                                             space="PSUM")
                proj_ps_open = True
                warm = tc.alloc_tile_pool(name="warmps", bufs=1,
                                          space="PSUM")
                warm_open = True

                def q_proj(sqb):
                    qt = qT_pool.tile([P, HC, SKB], FP8, tag="qT",
                                      name=f"qT_{sqb}")
                    qrt = qrT_pool.tile([P, HC, SKB], FP8, tag="qrT",
                                        name=f"qrT_{sqb}")
                    DG = DC // 2
                    for hc in range(HC):
                        ps = proj_ps.tile([P, SKB], F32, tag="proj_q")
                        hs = slice(hc * P, (hc + 1) * P)
                        streams = ((wq8_sb, xq_sb[sqb]),
                                   (wq8_sb, xqr_sb[sqb]),
                                   (wqr8_sb, xq_sb[sqb]))
                        nmm = len(streams) * DG
                        i = 0
                        for wt, xt in streams:
                            for g in range(DG):
                                nc.tensor.matmul(
                                    ps[:],
                                    wt[:, 2 * g:2 * g + 2, hs],
                                    xt[:, 2 * g:2 * g + 2, :],
                                    start=(i == 0), stop=(i == nmm - 1),
                                    perf_mode=mybir.MatmulPerfMode.DoubleRow)
                                i += 1
                        # psum = 64q (+ the tiny xr*wr tail of the last
                        # pair, below quantization noise); q48 = fp8(4q)
                        nc.vector.tensor_scalar_mul(qt[:, hc, :], ps[:],
                                                    1.0 / 16.0)
                        nc.vector.scalar_tensor_tensor(
                            qrt[:, hc, :], ps[:], 1.0 / 16.0, qt[:, hc, :],
                            op0=mybir.AluOpType.mult,
                            op1=mybir.AluOpType.subtract)
                    qT_sb[sqb] = qt
                    qrT_sb[sqb] = qrt

                # input staging: xq0 first (B-start gate via q_proj(0)),
                # then kT, the xq blocks for the interleaved q-projs,
                # first v pairs, mask sliver
                issue_xq(0)
                for skb in range(NSKB):
                    issue_kT(skb)
                issue_xq(1)
                issue_xq(2)
                for pr in range(4):
                    issue_v(pr)
                issue_mask_chunk(0, 0, split=True)
                q_proj(0)

                blk0_dma = {0: [("v", 4), ("v", 5)],
                            1: [("mask", 0, 1), ("xq", 3)],
                            2: [("v", 6), ("v", 7)],
                            3: [("mask", 0, 2)],
                            4: [("v", 8), ("v", 9)],
                            5: [("mask", 0, 3)],
                            6: [("v", 10), ("v", 11)],
                            8: [("v", 12), ("v", 13)],
                            10: [("v", 14), ("v", 15)],
                            11: [("mask", 1, 0)],
                            12: [("mask", 1, 1), ("mask", 1, 2)],
                            13: [("mask", 1, 3)]}

                for gt in range(NB):
                    if gt == LOOKP and warm_open:
                        warm.release()
                        warm_open = False
                        proj_ps.release()
                        proj_ps_open = False
                    if gt < NB:
                        sqb_s, ts = divmod(gt, NPAIR)
                        if sqb_s == 0:
                            for act in blk0_dma.get(ts, ()):
                                if act[0] == "v":
                                    issue_v(act[1])
                                elif act[0] == "xq":
                                    issue_xq(act[1])
                                else:
                                    issue_mask_chunk(act[1], act[2])
                            if ts == 1:
                                q_proj(1)
                            elif ts == 3:
                                q_proj(2)
                            elif ts == 5:
                                q_proj(3)
                        else:
                            if ts == 0 and sqb_s >= 2:
                                for g in range(NMG):
                                    issue_mask_chunk(sqb_s, g)
                            if ts == NPAIR // 2:
                                for g in range(NMG):
                                    issue_mask_chunk(sqb_s + 1, g)
                        pool_for_s2 = warm if gt in (1, 4) else s2_pool
                        s2 = pool_for_s2.tile([P, 2, SKB], F32, tag="s2",
                                              name=f"s2_{sqb_s}_{ts}")
                        for u in (0, 1):
                            kc = 2 * ts + u
                            skb, j = divmod(kc, SKB // P)
                            nc.tensor.matmul(
                                s2[:, u, :],
                                kT_sb[skb][:, :, j * P:(j + 1) * P],
                                qT_sb[sqb_s][:],
                                start=True, stop=False,
                                perf_mode=mybir.MatmulPerfMode.DoubleRow)
                            nc.tensor.matmul(
                                s2[:, u, :],
                                kT_sb[skb][:, :, j * P:(j + 1) * P],
                                qrT_sb[sqb_s][:],
                                start=False, stop=True,
                                perf_mode=mybir.MatmulPerfMode.DoubleRow)
                        e2 = e_pool.tile([P, 2, SKB], BF16, tag="e2")
                        nc.scalar.activation(
                            e2[:], s2[:], mybir.ActivationFunctionType.Exp,
                            scale=float(scale))
                        kc0 = 2 * ts
                        g0 = kc0 // MG
                        e1 = e1_pool.tile([P, 2, SKB], BF16, tag="e1")
                        nc.vector.tensor_scalar_sub(e1[:], e2[:], 1.0)
                        pt2 = pt_pool.tile([P, 2, SKB], FP8, tag="pt",
                                           name=f"pt2_{sqb_s}_{ts}")
                        nc.gpsimd.tensor_tensor(
                            pt2[:], e1[:],
                            m_chunks[(sqb_s, g0)][:, kc0 % MG:kc0 % MG + 2, :],
                            op=mybir.AluOpType.mult)
                        pts[gt] = pt2
                    gp = gt - LOOKP
                    if gp >= 0:
                        sqb_p, tp = divmod(gp, NPAIR)
                        if o_psum_pool is None:
                            o_psum_pool = tc.alloc_tile_pool(
                                name="opsum", bufs=SKB // P, space="PSUM")
                        if tp == 0:
                            o_ps_blk[sqb_p] = [
                                o_psum_pool.tile([P, H_ + 1], F32,
                                                 tag="opsum",
                                                 name=f"o_ps_{sqb_p}_{j2}")
                                for j2 in range(SKB // P)]
                        o_ps = o_ps_blk[sqb_p]
                        if tp == NPAIR - 1:
                            for j2 in range(SKB // P):
                                nc.tensor.matmul(
                                    o_ps[j2][:],
                                    pts[gp][:, :, j2 * P:(j2 + 1) * P],
                                    vv8_sb[tp][:, :, 0, :],
                                    start=(tp == 0), stop=False,
                                    perf_mode=mybir.MatmulPerfMode.DoubleRow)
                                nc.tensor.matmul(
                                    o_ps[j2][:],
                                    pts[gp][:, :, j2 * P:(j2 + 1) * P],
                                    vv8_sb[tp][:, :, 1, :],
                                    start=False, stop=True,
                                    perf_mode=mybir.MatmulPerfMode.DoubleRow)
                                o_sb = o_sb_pool.tile([P, H_ + 1], F32,
                                                      tag="osb")
                                nc.vector.tensor_copy(o_sb[:], o_ps[j2][:])
                                r0 = sqb_p * SKB + j2 * P
                                nc.sync.dma_start(out=out[r0:r0 + P, :],
                                                  in_=o_sb[:])
                            del pts[gp]
                        else:
                            for j2 in range(SKB // P):
                                nc.tensor.matmul(
                                    o_ps[j2][:],
                                    pts[gp][:, :, j2 * P:(j2 + 1) * P],
                                    vv8_sb[tp][:, :, 0, :],
                                    start=(tp == 0), stop=False,
                                    perf_mode=mybir.MatmulPerfMode.DoubleRow)
                                nc.tensor.matmul(
                                    o_ps[j2][:],
                                    pts[gp][:, :, j2 * P:(j2 + 1) * P],
                                    vv8_sb[tp][:, :, 1, :],
                                    start=False, stop=False,
                                    perf_mode=mybir.MatmulPerfMode.DoubleRow)
                            del pts[gp]
                # drain: the final block's last LOOKP pairs, j2-major so
                # each accumulator stops (and its copy/DMA issues) while
                # the next j2 chain still runs on the PE
                if (NSQB - 1) not in o_ps_blk:
                    o_ps_blk[NSQB - 1] = [
                        o_psum_pool.tile([P, H_ + 1], F32, tag="opsum",
                                         name=f"o_ps_{NSQB - 1}_{j2}")
                        for j2 in range(SKB // P)]
                o_ps = o_ps_blk[NSQB - 1]
                for j2 in range(SKB // P):
                    for gp in range(NB - LOOKP, NB):
                        tp = gp % NPAIR
                        nc.tensor.matmul(
                            o_ps[j2][:],
                            pts[gp][:, :, j2 * P:(j2 + 1) * P],
                            vv8_sb[tp][:, :, 0, :],
                            start=(tp == 0), stop=False,
                            perf_mode=mybir.MatmulPerfMode.DoubleRow)
                        nc.tensor.matmul(
                            o_ps[j2][:],
                            pts[gp][:, :, j2 * P:(j2 + 1) * P],
                            vv8_sb[tp][:, :, 1, :],
                            start=False, stop=(tp == NPAIR - 1),
                            perf_mode=mybir.MatmulPerfMode.DoubleRow)
                    o_sb = o_sb_pool.tile([P, H_ + 1], F32, tag="osb")
                    nc.vector.tensor_copy(o_sb[:], o_ps[j2][:])
                    r0 = (NSQB - 1) * SKB + j2 * P
                    nc.sync.dma_start(out=out[r0:r0 + P, :], in_=o_sb[:])
                o_psum_pool.release()

    nc.compile()
    return nc


_L1 = None
_L2 = None

TRACE = False
LAST_RESULT = None


def _get_l1():
    global _L1
    if _L1 is None:
        _L1 = build_kv_nc(SKH, DV, H, num_devices=N_CORES)
    return _L1


def _get_l2():
    global _L2
    if _L2 is None:
        _L2 = build_attn_nc(SQL, S, DV, H, scale=1.0 / 16.0,
                            num_devices=N_CORES)
    return _L2


def kernel(x_q, x_k, x_v, mask, wq_w, wq_b, wk_w, wk_b, wv_w, wv_b):
    to_bf = lambda a: np.asarray(a, np.float32).astype(ml_dtypes.bfloat16)
    xkT = np.ascontiguousarray(np.swapaxes(to_bf(x_k), 1, 2))
    xvT = np.ascontiguousarray(np.swapaxes(to_bf(x_v), 1, 2))
    maskT = np.ascontiguousarray(np.swapaxes(
        np.asarray(mask).astype(ml_dtypes.float8_e4m3), 1, 2))
    # compensated-fp8 q path: x = x8 + xr8, w (scaled 64x) = w8 + wr8;
    # psum accumulates 64q, rescaled to 4q at the q48 store
    fp8t = ml_dtypes.float8_e4m3
    xq_f = np.asarray(x_q, np.float32)
    xq8 = xq_f.astype(fp8t)
    xqr8 = (xq_f - xq8.astype(np.float32)).astype(fp8t)
    xq8T = np.ascontiguousarray(np.swapaxes(xq8, 1, 2))    # [B, DV, S]
    xqr8T = np.ascontiguousarray(np.swapaxes(xqr8, 1, 2))
    wq64 = 64.0 * np.asarray(wq_w, np.float32)
    wq8 = wq64.astype(fp8t)
    wqr8 = (wq64 - wq8.astype(np.float32)).astype(fp8t)
    wq8T = np.ascontiguousarray(wq8.T)                     # [DV, H]
    wqr8T = np.ascontiguousarray(wqr8.T)
    wkT = np.ascontiguousarray(to_bf(wk_w).T)
    wvT = np.ascontiguousarray(to_bf(wv_w).T)

    # ---- launch 1: k/v projections on key halves ----
    l1_maps = []
    for c in range(N_CORES):
        b, h = divmod(c, CORES_PER_BATCH)
        k0 = h * SKH
        l1_maps.append({
            "x_kTh": np.ascontiguousarray(xkT[b][:, k0:k0 + SKH]),
            "x_vTh": np.ascontiguousarray(xvT[b][:, k0:k0 + SKH]),
            "wkT": wkT,
            "wvT": wvT,
        })
    res1 = run_bass_kernel_spmd(_get_l1(), l1_maps,
                                core_ids=list(range(N_CORES)), trace=False)
    o1 = res1.results

    # host exchange: concat halves per batch
    kT_full = [np.concatenate([o1[2 * b]["kTh"], o1[2 * b + 1]["kTh"]],
                              axis=1) for b in range(B)]
    vv8_full = [np.concatenate([o1[2 * b]["vv8h"], o1[2 * b + 1]["vv8h"]],
                               axis=0) for b in range(B)]

    # ---- launch 2: q-projection + attention ----
    l2_maps = []
    for c in range(N_CORES):
        b, half = divmod(c, CORES_PER_BATCH)
        q0 = half * SQL
        l2_maps.append({
            "xq8T": np.ascontiguousarray(xq8T[b][:, q0:q0 + SQL]),
            "xqr8T": np.ascontiguousarray(xqr8T[b][:, q0:q0 + SQL]),
            "kT_in": kT_full[b],
            "vv8_in": vv8_full[b],
            "maskT": np.ascontiguousarray(maskT[b][:, q0:q0 + SQL]),
            "wq8T": wq8T,
            "wqr8T": wqr8T,
        })
    global LAST_RESULT
    res2 = run_bass_kernel_spmd(_get_l2(), l2_maps,
                                core_ids=list(range(N_CORES)), trace=TRACE)
    LAST_RESULT = res2
    o2 = res2.results

    # host finish: out = (raw[:, :H] + colsum(v_eff)) / (raw[:, H] + S)
    # v_eff comes straight from the chip's v8+vr8 tensors
    full = np.empty((B, S, H), dtype=np.float32)
    for bidx in range(B):
        vv = vv8_full[bidx].astype(np.float32)   # [NPR*P, 2, 2, 257]
        v_eff = vv[:, :, 0, :] + vv[:, :, 1, :]  # [NPR*P, 2, 257]
        colsum = v_eff[:, :, :H].astype(np.float64).sum(axis=(0, 1))
        for half in range(CORES_PER_BATCH):
            c = bidx * CORES_PER_BATCH + half
            raw = np.asarray(o2[c]["out"], np.float64)
            q0 = half * SQL
            num = raw[:, :H] + colsum[None, :]
            den = raw[:, H:H + 1] + float(S)
            full[bidx, q0:q0 + SQL] = (num / den).astype(np.float32)
    return full


# revision 53
# speedup vs baseline: 1.0060x; 1.0060x over previous
"""Trainium2 Bass kernel for nn_AttentionHead (sparse attention, 8 cores).

Reference computation (per batch b):
    q = x_q @ wq^T ; k = x_k @ wk^T ; v = x_v @ wv^T          # [S, H]
    s = (q @ k^T) / sqrt(H)                                    # [S, S]
    s = where(mask == 0, 0, s)       # multiplicative 0/1 mask BEFORE softmax
    p = softmax(s, axis=-1)          # masked entries contribute exp(0)=1
    out = p @ v                                                # [S, H]

Sharding: 8 cores; core c -> batch c//2.  TWO launches:
  L1: core c projects k and v for key half c%2 only (removing the k/v
      projection duplication between the two cores of a batch) and ships
      kT = fp8(k/4) plus the compensated value pair v8 = fp8(v),
      vr8 = fp8(v - v8) back to DRAM; the host concatenates the halves.
  L2: core c computes attention for query rows (c%2)*2048 ... +2048
      against all 4096 keys.

All heavy matmuls run fp8 DoubleRow (0.5 cyc/row, K=256/instruction)
with error compensation to hold the 2e-2 gate:
  scores = k4s8 @ (q48 + qr48): k4s8 = fp8(k/4) (single shared tile),
    q48 = fp8(4q) (wq host-scaled by 4 so the PSUM is 4q), qr48 =
    fp8(4q - q48).  The residual removes the q-side fp8 error; the
    k-side ~2.9% remains -> ~1.3e-2 output contribution.
  P = m*(E-1) + 1 identity: the chip computes raw = sum m*(E-1)*v_ext;
    fp8((E-1)*m) has 2.4x less quantization error than fp8(E*m) since
    rms(E-1) = 0.82 vs rms(E) = 1.44.  Host finish:
    out = (raw[:, :H] + colsum(v_eff)) / (raw[:, H] + S).
  PV = pt'8^T @ (v8 + vr8): two fp8-DR matmuls into one accumulator.
  pt' path: ACT exp (f32 PSUM -> bf16), DVE tensor_scalar_sub (E-1, 2x
    rate), Pool tensor_tensor multiply with the fp8 mask -> fp8
    (scalar_tensor_tensor is not ISA-legal on Pool; this split is).
  q-projection runs compensated fp8-DR: x_q = x8 + xr8 (two fp8
    tensors, same bytes as bf16), wq (host-scaled 64x) = w8 + wr8;
    psum = x8@w8 + xr8@w8 + x8@wr8 = 64q in three fp8-DR streams at
    75% of the bf16 matmul cost, and ~2x LESS input-quantization error
    than bf16 x/w (residuals bound the error at ~0.1%).  The xr8@wr8
    term is dropped (~0.03% of q, below quantization noise).
  k/v projections stay bf16 (L1 is DMA-bound; fp8-DR would not help).

Pipeline notes: L2 keeps a 1-buf "warm" PSUM score pool alive until
the PV accumulators take its banks at gt = LOOKP, giving the exp
stream a third score buffer across the q-projection interruptions.
LOOKP = 16 (a full block of pts lag): each +2 of lag removed ~0.9us of
per-block PV/score boundary coupling; the final block's PV runs
entirely in a j2-major drain whose output copies overlap the chains.

CoreSim cost-model: L1 39.3us + L2 88.4us = 127.7us/core (baseline
149.2us).  HW-validated rel err 1.812e-2 (gate 2e-2), deterministic.
"""

import numpy as np
import ml_dtypes

import concourse.bass as bass
import concourse.mybir as mybir
import concourse.tile as tile
from concourse import bacc
from concourse.bass_utils import run_bass_kernel_spmd

F32 = mybir.dt.float32
BF16 = mybir.dt.bfloat16
FP8 = mybir.dt.float8e4

B, S, DV, H = 4, 4096, 1024, 256
N_CORES = 8
CORES_PER_BATCH = N_CORES // B
SQL = S // CORES_PER_BATCH
SKH = S // 2                  # keys per core in L1

QSC = 4.0


def build_kv_nc(SKH_, DV_, H_, num_devices=1):
    """L1: kTh = fp8(k/4) [H, SKH]; v8h/vr8h [SKH/2, 2, H+1] fp8
    (row r = pr*128+p, slot u -> sk = pr*256 + u*128 + p)."""
    P = 128
    SKB = 512
    DC = DV_ // P
    NSKB = SKH_ // SKB
    HC = H_ // P
    NPR = SKH_ // 256             # v pair tiles

    nc = bacc.Bacc("TRN2", target_bir_lowering=False, debug=False,
                   num_devices=num_devices)

    x_kT = nc.dram_tensor("x_kTh", [DV_, SKH_], BF16, kind="ExternalInput").ap()
    x_vT = nc.dram_tensor("x_vTh", [DV_, SKH_], BF16, kind="ExternalInput").ap()
    wkT = nc.dram_tensor("wkT", [DV_, H_], BF16, kind="ExternalInput").ap()
    wvT = nc.dram_tensor("wvT", [DV_, H_], BF16, kind="ExternalInput").ap()
    kTh = nc.dram_tensor("kTh", [H_, SKH_], FP8, kind="ExternalOutput").ap()
    # v8 and vr8 interleaved in one tensor: vv8h[r, u, 0, :] = v8,
    # vv8h[r, u, 1, :] = vr8 -> one DMA per pair, 514+B descriptors
    vv8h = nc.dram_tensor("vv8h", [NPR * P, 2, 2, H_ + 1], FP8,
                          kind="ExternalOutput").ap()

    with tile.TileContext(nc) as tc:
        with (
            tc.tile_pool(name="weights", bufs=2) as w_pool,
            tc.tile_pool(name="xT", bufs=3) as xT_pool,
            tc.tile_pool(name="kt", bufs=3) as kt_pool,
            tc.tile_pool(name="v8", bufs=4) as v8_pool,
            tc.tile_pool(name="kps", bufs=2, space="PSUM") as k_ps,
            tc.tile_pool(name="vps", bufs=2, space="PSUM") as v_ps,
        ):
            w_sb = {}

            def load_w(name, wT, split=False):
                t = w_pool.tile([P, DC, H_], BF16, tag=f"w_{name}",
                                name=f"w_{name}")
                src_ap = wT.rearrange("(dc p) h -> p dc h", p=P)
                if split:
                    nc.sync.dma_start(out=t[:, 0:1, :], in_=src_ap[:, 0:1, :])
                    nc.sync.dma_start(out=t[:, 1:DC, :], in_=src_ap[:, 1:DC, :])
                else:
                    nc.sync.dma_start(out=t[:], in_=src_ap)
                w_sb[name] = t

            load_w("k", wkT, split=True)

            xv_sb = [None] * NSKB

            def issue_xv(skb):
                if skb >= NSKB or xv_sb[skb] is not None:
                    return
                t = xT_pool.tile([P, DC, SKB], BF16, tag="xvT",
                                 name=f"xv_{skb}")
                nc.sync.dma_start(
                    out=t[:],
                    in_=x_vT[:, skb * SKB:(skb + 1) * SKB].rearrange(
                        "(dc p) n -> p dc n", p=P))
                xv_sb[skb] = t

            def k_block(skb):
                xk_t = xT_pool.tile([P, DC, SKB], BF16, tag="xkT",
                                    name=f"xk_{skb}")
                xk_src = x_kT[:, skb * SKB:(skb + 1) * SKB].rearrange(
                    "(dc p) n -> p dc n", p=P)
                if skb == 0:
                    for a, b in ((0, 1), (1, 2), (2, 4), (4, DC)):
                        nc.sync.dma_start(out=xk_t[:, a:b, :],
                                          in_=xk_src[:, a:b, :])
                else:
                    nc.sync.dma_start(out=xk_t[:], in_=xk_src)
                kt = kt_pool.tile([P, HC, SKB], FP8, tag="kT",
                                  name=f"kT_{skb}")
                for hc in range(HC):
                    ps = k_ps.tile([P, SKB], F32, tag="kps")
                    for dc in range(DC):
                        nc.tensor.matmul(
                            ps[:],
                            w_sb["k"][:, dc, hc * P:(hc + 1) * P],
                            xk_t[:, dc, :],
                            start=(dc == 0), stop=(dc == DC - 1))
                    nc.vector.tensor_scalar_mul(kt[:, hc, :], ps[:],
                                                1.0 / QSC)
                nc.scalar.dma_start(
                    out=kTh[:, skb * SKB:(skb + 1) * SKB].rearrange(
                        "(hc p) n -> p hc n", p=P),
                    in_=kt[:])

            def v_pair(pr):
                vvt = v8_pool.tile([P, 2, 2, H_ + 1], FP8, tag="vv8",
                                   name=f"vv8_{pr}")
                for u in (0, 1):
                    kc = 2 * pr + u
                    skb, j = divmod(kc, SKB // P)
                    ps = v_ps.tile([P, H_], F32, tag="vps")
                    for dc in range(DC):
                        nc.tensor.matmul(
                            ps[:],
                            xv_sb[skb][:, dc, j * P:(j + 1) * P],
                            w_sb["v"][:, dc, :],
                            start=(dc == 0), stop=(dc == DC - 1))
                    nc.vector.tensor_copy(vvt[:, u, 0, 0:H_], ps[:])
                    nc.vector.scalar_tensor_tensor(
                        vvt[:, u, 1, 0:H_], ps[:], 1.0, vvt[:, u, 0, 0:H_],
                        op0=mybir.AluOpType.mult,
                        op1=mybir.AluOpType.subtract)
                nc.gpsimd.memset(vvt[:, :, 0, H_:H_ + 1], 1.0)
                nc.gpsimd.memset(vvt[:, :, 1, H_:H_ + 1], 0.0)
                # out-DMA issued from the (otherwise idle) ACT queue so
                # the SP queue only carries the input stream
                nc.scalar.dma_start(out=vv8h[pr * P:(pr + 1) * P, :, :, :],
                                    in_=vvt[:])

            # interleave k and v blocks: they are independent, so the
            # PE alternates while the bus streams xk/xv back to back
            load_w("v", wvT)
            for skb in range(NSKB):
                k_block(skb)
                issue_xv(skb)
                v_pair(2 * skb)
                v_pair(2 * skb + 1)

    nc.compile()
    return nc


def build_attn_nc(SQL_, SK_, DV_, H_, scale, num_devices=1):
    """L2: q-projection + attention; kT/v8/vr8 come from DRAM (L1)."""
    P = 128
    SKB = 512
    DC = DV_ // P
    NSKB = SK_ // SKB
    NKC = SK_ // P
    NSQB = SQL_ // SKB
    HC = H_ // P
    NPAIR = NKC // 2
    NPRT = SK_ // 256             # v pair tiles total
    LOOKP = 16

    nc = bacc.Bacc("TRN2", target_bir_lowering=False, debug=False,
                   num_devices=num_devices)

    xq8T = nc.dram_tensor("xq8T", [DV_, SQL_], FP8, kind="ExternalInput").ap()
    xqr8T = nc.dram_tensor("xqr8T", [DV_, SQL_], FP8,
                           kind="ExternalInput").ap()
    kT_in = nc.dram_tensor("kT_in", [H_, SK_], FP8, kind="ExternalInput").ap()
    vv8_in = nc.dram_tensor("vv8_in", [NPRT * P, 2, 2, H_ + 1], FP8,
                            kind="ExternalInput").ap()
    maskT = nc.dram_tensor("maskT", [SK_, SQL_], FP8, kind="ExternalInput").ap()
    wq8T = nc.dram_tensor("wq8T", [DV_, H_], FP8, kind="ExternalInput").ap()
    wqr8T = nc.dram_tensor("wqr8T", [DV_, H_], FP8, kind="ExternalInput").ap()
    out = nc.dram_tensor("out", [SQL_, H_ + 1], F32, kind="ExternalOutput").ap()

    with tile.TileContext(nc) as tc:
        with (
            tc.tile_pool(name="weights", bufs=1) as w_pool,
            tc.tile_pool(name="qT", bufs=NSQB) as qT_pool,
            tc.tile_pool(name="qrT", bufs=NSQB) as qrT_pool,
            tc.tile_pool(name="kT", bufs=NSKB) as kT_pool,
            tc.tile_pool(name="v8", bufs=NPRT) as v8_pool,
            tc.tile_pool(name="maskp", bufs=7) as mask_pool,
            tc.tile_pool(name="xq", bufs=4) as xq_pool,
        ):
            # q weights first, then xq0 — these gate q_proj(0) and thus
            # the whole B pipeline; kT tiles stream in behind them.
            # Compensated-fp8 q path: psum accumulates
            # (x8+xr8)@w8 + x8@wr8 = 64q (w host-scaled by 64), three
            # fp8-DR streams at 75% of the bf16 projection cost.
            wq8_sb = w_pool.tile([P, DC, H_], FP8, tag="w_q8", name="w_q8")
            nc.sync.dma_start(out=wq8_sb[:],
                              in_=wq8T.rearrange("(dc p) h -> p dc h", p=P))
            wqr8_sb = w_pool.tile([P, DC, H_], FP8, tag="w_qr8",
                                  name="w_qr8")
            nc.sync.dma_start(out=wqr8_sb[:],
                              in_=wqr8T.rearrange("(dc p) h -> p dc h", p=P))

            # double-wide kT tiles: 4 DMAs of 0.25 MiB instead of 8,
            # halving the SP issues queued ahead of xq1 in the B-start
            kT_sb = [None] * (NSKB // 2)

            def issue_kT(g):
                if g >= NSKB // 2 or kT_sb[g] is not None:
                    return
                t = kT_pool.tile([P, HC, 2 * SKB], FP8, tag="kT",
                                 name=f"kT_{g}")
                nc.sync.dma_start(
                    out=t[:],
                    in_=kT_in[:, g * 2 * SKB:(g + 1) * 2 * SKB].rearrange(
                        "(hc p) n -> p hc n", p=P))
                kT_sb[g] = t

            vv8_sb = [None] * NPRT

            def issue_v(pr):
                if pr >= NPRT or vv8_sb[pr] is not None:
                    return
                t = v8_pool.tile([P, 2, 2, H_ + 1], FP8, tag="vv8",
                                 name=f"vv8_{pr}")
                nc.sync.dma_start(out=t[:],
                                  in_=vv8_in[pr * P:(pr + 1) * P, :, :, :])
                vv8_sb[pr] = t

            xq_sb = [None] * NSQB
            xqr_sb = [None] * NSQB

            def issue_xq(sqb):
                if sqb >= NSQB or xq_sb[sqb] is not None:
                    return
                t = xq_pool.tile([P, DC, SKB], FP8, tag="xq",
                                 name=f"xq_{sqb}")
                src = xq8T[:, sqb * SKB:(sqb + 1) * SKB].rearrange(
                    "(dc p) n -> p dc n", p=P)
                tr = xq_pool.tile([P, DC, SKB], FP8, tag="xqr",
                                  name=f"xqr_{sqb}")
                srcr = xqr8T[:, sqb * SKB:(sqb + 1) * SKB].rearrange(
                    "(dc p) n -> p dc n", p=P)
                if sqb == 0:
                    for a, b in ((0, 2), (2, 4), (4, DC)):
                        nc.sync.dma_start(out=t[:, a:b, :], in_=src[:, a:b, :])
                else:
                    nc.sync.dma_start(out=t[:], in_=src)
                nc.sync.dma_start(out=tr[:], in_=srcr)
                xq_sb[sqb] = t
                xqr_sb[sqb] = tr

            MG = 8
            NMG = NKC // MG
            m_chunks = {}

            def issue_mask_chunk(sqb, g, split=False):
                if (sqb, g) in m_chunks or sqb >= NSQB:
                    return
                t = mask_pool.tile([P, MG, SKB], FP8, tag="maskT",
                                   name=f"mask_{sqb}_{g}")
                src_ap = maskT[g * MG * P:(g + 1) * MG * P,
                               sqb * SKB:(sqb + 1) * SKB].rearrange(
                                   "(kc p) n -> p kc n", p=P)
                if split:
                    nc.sync.dma_start(out=t[:, 0:2, :], in_=src_ap[:, 0:2, :])
                    nc.sync.dma_start(out=t[:, 2:MG, :], in_=src_ap[:, 2:MG, :])
                else:
                    nc.sync.dma_start(out=t[:], in_=src_ap)
                m_chunks[(sqb, g)] = t

            qT_sb = [None] * NSQB
            qrT_sb = [None] * NSQB

            NB = NSQB * NPAIR
            o_ps_blk = {}
            pts = {}
            o_psum_pool = None
            with (
                tc.tile_pool(name="ep", bufs=4) as e_pool,
                tc.tile_pool(name="e1p", bufs=4) as e1_pool,
                tc.tile_pool(name="ptp", bufs=LOOKP + 3) as pt_pool,
                tc.tile_pool(name="osb", bufs=6) as o_sb_pool,
                tc.tile_pool(name="s2psum", bufs=2, space="PSUM") as s2_pool,
            ):
                proj_ps = tc.alloc_tile_pool(name="projps", bufs=2,
                                             space="PSUM")
                proj_ps_open = True
                warm = tc.alloc_tile_pool(name="warmps", bufs=1,
                                          space="PSUM")
                warm_open = True

                def q_proj(sqb):
                    qt = qT_pool.tile([P, HC, SKB], FP8, tag="qT",
                                      name=f"qT_{sqb}")
                    qrt = qrT_pool.tile([P, HC, SKB], FP8, tag="qrT",
                                        name=f"qrT_{sqb}")
                    DG = DC // 2
                    for hc in range(HC):
                        ps = proj_ps.tile([P, SKB], F32, tag="proj_q")
                        hs = slice(hc * P, (hc + 1) * P)
                        streams = ((wq8_sb, xq_sb[sqb]),
                                   (wq8_sb, xqr_sb[sqb]),
                                   (wqr8_sb, xq_sb[sqb]))
                        nmm = len(streams) * DG
                        i = 0
                        for wt, xt in streams:
                            for g in range(DG):
                                nc.tensor.matmul(
                                    ps[:],
                                    wt[:, 2 * g:2 * g + 2, hs],
                                    xt[:, 2 * g:2 * g + 2, :],
                                    start=(i == 0), stop=(i == nmm - 1),
                                    perf_mode=mybir.MatmulPerfMode.DoubleRow)
                                i += 1
                        # psum = 64q (+ the tiny xr*wr tail of the last
                        # pair, below quantization noise); q48 = fp8(4q)
                        nc.vector.tensor_scalar_mul(qt[:, hc, :], ps[:],
                                                    1.0 / 16.0)
                        nc.vector.scalar_tensor_tensor(
                            qrt[:, hc, :], ps[:], 1.0 / 16.0, qt[:, hc, :],
                            op0=mybir.AluOpType.mult,
                            op1=mybir.AluOpType.subtract)
                    qT_sb[sqb] = qt
                    qrT_sb[sqb] = qrt

                # input staging: xq0 first (B-start gate via q_proj(0)),
                # then kT, the xq blocks for the interleaved q-projs,
                # first v pairs, mask sliver
                issue_xq(0)
                for g in range(NSKB // 2):
                    issue_kT(g)
                issue_xq(1)
                issue_xq(2)
                for pr in range(4):
                    issue_v(pr)
                issue_mask_chunk(0, 0, split=True)
                q_proj(0)

                blk0_dma = {0: [("v", 4), ("v", 5)],
                            1: [("mask", 0, 1), ("xq", 3)],
                            2: [("v", 6), ("v", 7)],
                            3: [("mask", 0, 2)],
                            4: [("v", 8), ("v", 9)],
                            5: [("mask", 0, 3)],
                            6: [("v", 10), ("v", 11)],
                            8: [("v", 12), ("v", 13)],
                            10: [("v", 14), ("v", 15)],
                            11: [("mask", 1, 0)],
                            12: [("mask", 1, 1), ("mask", 1, 2)],
                            13: [("mask", 1, 3)]}

                for gt in range(NB):
                    if gt == LOOKP and warm_open:
                        warm.release()
                        warm_open = False
                        proj_ps.release()
                        proj_ps_open = False
                    if gt < NB:
                        sqb_s, ts = divmod(gt, NPAIR)
                        if sqb_s == 0:
                            for act in blk0_dma.get(ts, ()):
                                if act[0] == "v":
                                    issue_v(act[1])
                                elif act[0] == "xq":
                                    issue_xq(act[1])
                                else:
                                    issue_mask_chunk(act[1], act[2])
                            if ts == 1:
                                q_proj(1)
                            elif ts == 3:
                                q_proj(2)
                            elif ts == 5:
                                q_proj(3)
                        else:
                            if ts == 0 and sqb_s >= 2:
                                for g in range(NMG):
                                    issue_mask_chunk(sqb_s, g)
                            if ts == NPAIR // 2:
                                for g in range(NMG):
                                    issue_mask_chunk(sqb_s + 1, g)
                        pool_for_s2 = warm if gt in (1, 4) else s2_pool
                        s2 = pool_for_s2.tile([P, 2, SKB], F32, tag="s2",
                                              name=f"s2_{sqb_s}_{ts}")
                        for u in (0, 1):
                            kc = 2 * ts + u
                            g2, jj = divmod(kc, 2 * SKB // P)
                            nc.tensor.matmul(
                                s2[:, u, :],
                                kT_sb[g2][:, :, jj * P:(jj + 1) * P],
                                qT_sb[sqb_s][:],
                                start=True, stop=False,
                                perf_mode=mybir.MatmulPerfMode.DoubleRow)
                            nc.tensor.matmul(
                                s2[:, u, :],
                                kT_sb[g2][:, :, jj * P:(jj + 1) * P],
                                qrT_sb[sqb_s][:],
                                start=False, stop=True,
                                perf_mode=mybir.MatmulPerfMode.DoubleRow)
                        e2 = e_pool.tile([P, 2, SKB], BF16, tag="e2")
                        nc.scalar.activation(
                            e2[:], s2[:], mybir.ActivationFunctionType.Exp,
                            scale=float(scale))
                        kc0 = 2 * ts
                        g0 = kc0 // MG
                        e1 = e1_pool.tile([P, 2, SKB], BF16, tag="e1")
                        nc.vector.tensor_scalar_sub(e1[:], e2[:], 1.0)
                        pt2 = pt_pool.tile([P, 2, SKB], FP8, tag="pt",
                                           name=f"pt2_{sqb_s}_{ts}")
                        nc.gpsimd.tensor_tensor(
                            pt2[:], e1[:],
                            m_chunks[(sqb_s, g0)][:, kc0 % MG:kc0 % MG + 2, :],
                            op=mybir.AluOpType.mult)
                        pts[gt] = pt2
                    gp = gt - LOOKP
                    if gp >= 0:
                        sqb_p, tp = divmod(gp, NPAIR)
                        if o_psum_pool is None:
                            o_psum_pool = tc.alloc_tile_pool(
                                name="opsum", bufs=SKB // P, space="PSUM")
                        if tp == 0:
                            o_ps_blk[sqb_p] = [
                                o_psum_pool.tile([P, H_ + 1], F32,
                                                 tag="opsum",
                                                 name=f"o_ps_{sqb_p}_{j2}")
                                for j2 in range(SKB // P)]
                        o_ps = o_ps_blk[sqb_p]
                        if tp == NPAIR - 1:
                            for j2 in range(SKB // P):
                                nc.tensor.matmul(
                                    o_ps[j2][:],
                                    pts[gp][:, :, j2 * P:(j2 + 1) * P],
                                    vv8_sb[tp][:, :, 0, :],
                                    start=(tp == 0), stop=False,
                                    perf_mode=mybir.MatmulPerfMode.DoubleRow)
                                nc.tensor.matmul(
                                    o_ps[j2][:],
                                    pts[gp][:, :, j2 * P:(j2 + 1) * P],
                                    vv8_sb[tp][:, :, 1, :],
                                    start=False, stop=True,
                                    perf_mode=mybir.MatmulPerfMode.DoubleRow)
                                o_sb = o_sb_pool.tile([P, H_ + 1], F32,
                                                      tag="osb")
                                nc.vector.tensor_copy(o_sb[:], o_ps[j2][:])
                                r0 = sqb_p * SKB + j2 * P
                                nc.sync.dma_start(out=out[r0:r0 + P, :],
                                                  in_=o_sb[:])
                            del pts[gp]
                        else:
                            for j2 in range(SKB // P):
                                nc.tensor.matmul(
                                    o_ps[j2][:],
                                    pts[gp][:, :, j2 * P:(j2 + 1) * P],
                                    vv8_sb[tp][:, :, 0, :],
                                    start=(tp == 0), stop=False,
                                    perf_mode=mybir.MatmulPerfMode.DoubleRow)
                                nc.tensor.matmul(
                                    o_ps[j2][:],
                                    pts[gp][:, :, j2 * P:(j2 + 1) * P],
                                    vv8_sb[tp][:, :, 1, :],
                                    start=False, stop=False,
                                    perf_mode=mybir.MatmulPerfMode.DoubleRow)
                            del pts[gp]
                # drain: the final block's last LOOKP pairs, j2-major so
                # each accumulator stops (and its copy/DMA issues) while
                # the next j2 chain still runs on the PE
                if (NSQB - 1) not in o_ps_blk:
                    o_ps_blk[NSQB - 1] = [
                        o_psum_pool.tile([P, H_ + 1], F32, tag="opsum",
                                         name=f"o_ps_{NSQB - 1}_{j2}")
                        for j2 in range(SKB // P)]
                o_ps = o_ps_blk[NSQB - 1]
                for j2 in range(SKB // P):
                    for gp in range(NB - LOOKP, NB):
                        tp = gp % NPAIR
                        nc.tensor.matmul(
                            o_ps[j2][:],
                            pts[gp][:, :, j2 * P:(j2 + 1) * P],
                            vv8_sb[tp][:, :, 0, :],
                            start=(tp == 0), stop=False,
                            perf_mode=mybir.MatmulPerfMode.DoubleRow)
                        nc.tensor.matmul(
                            o_ps[j2][:],
                            pts[gp][:, :, j2 * P:(j2 + 1) * P],
                            vv8_sb[tp][:, :, 1, :],
                            start=False, stop=(tp == NPAIR - 1),
                            perf_mode=mybir.MatmulPerfMode.DoubleRow)
                    o_sb = o_sb_pool.tile([P, H_ + 1], F32, tag="osb")
                    nc.vector.tensor_copy(o_sb[:], o_ps[j2][:])
                    r0 = (NSQB - 1) * SKB + j2 * P
                    nc.sync.dma_start(out=out[r0:r0 + P, :], in_=o_sb[:])
                o_psum_pool.release()

    nc.compile()
    return nc


_L1 = None
_L2 = None

TRACE = False
LAST_RESULT = None


def _get_l1():
    global _L1
    if _L1 is None:
        _L1 = build_kv_nc(SKH, DV, H, num_devices=N_CORES)
    return _L1


def _get_l2():
    global _L2
    if _L2 is None:
        _L2 = build_attn_nc(SQL, S, DV, H, scale=1.0 / 16.0,
                            num_devices=N_CORES)
    return _L2


def kernel(x_q, x_k, x_v, mask, wq_w, wq_b, wk_w, wk_b, wv_w, wv_b):
    to_bf = lambda a: np.asarray(a, np.float32).astype(ml_dtypes.bfloat16)
    xkT = np.ascontiguousarray(np.swapaxes(to_bf(x_k), 1, 2))
    xvT = np.ascontiguousarray(np.swapaxes(to_bf(x_v), 1, 2))
    maskT = np.ascontiguousarray(np.swapaxes(
        np.asarray(mask).astype(ml_dtypes.float8_e4m3), 1, 2))
    # compensated-fp8 q path: x = x8 + xr8, w (scaled 64x) = w8 + wr8;
    # psum accumulates 64q, rescaled to 4q at the q48 store
    fp8t = ml_dtypes.float8_e4m3
    xq_f = np.asarray(x_q, np.float32)
    xq8 = xq_f.astype(fp8t)
    xqr8 = (xq_f - xq8.astype(np.float32)).astype(fp8t)
    xq8T = np.ascontiguousarray(np.swapaxes(xq8, 1, 2))    # [B, DV, S]
    xqr8T = np.ascontiguousarray(np.swapaxes(xqr8, 1, 2))
    wq64 = 64.0 * np.asarray(wq_w, np.float32)
    wq8 = wq64.astype(fp8t)
    wqr8 = (wq64 - wq8.astype(np.float32)).astype(fp8t)
    wq8T = np.ascontiguousarray(wq8.T)                     # [DV, H]
    wqr8T = np.ascontiguousarray(wqr8.T)
    wkT = np.ascontiguousarray(to_bf(wk_w).T)
    wvT = np.ascontiguousarray(to_bf(wv_w).T)

    # ---- launch 1: k/v projections on key halves ----
    l1_maps = []
    for c in range(N_CORES):
        b, h = divmod(c, CORES_PER_BATCH)
        k0 = h * SKH
        l1_maps.append({
            "x_kTh": np.ascontiguousarray(xkT[b][:, k0:k0 + SKH]),
            "x_vTh": np.ascontiguousarray(xvT[b][:, k0:k0 + SKH]),
            "wkT": wkT,
            "wvT": wvT,
        })
    res1 = run_bass_kernel_spmd(_get_l1(), l1_maps,
                                core_ids=list(range(N_CORES)), trace=False)
    o1 = res1.results

    # host exchange: concat halves per batch
    kT_full = [np.concatenate([o1[2 * b]["kTh"], o1[2 * b + 1]["kTh"]],
                              axis=1) for b in range(B)]
    vv8_full = [np.concatenate([o1[2 * b]["vv8h"], o1[2 * b + 1]["vv8h"]],
                               axis=0) for b in range(B)]

    # ---- launch 2: q-projection + attention ----
    l2_maps = []
    for c in range(N_CORES):
        b, half = divmod(c, CORES_PER_BATCH)
        q0 = half * SQL
        l2_maps.append({
            "xq8T": np.ascontiguousarray(xq8T[b][:, q0:q0 + SQL]),
            "xqr8T": np.ascontiguousarray(xqr8T[b][:, q0:q0 + SQL]),
            "kT_in": kT_full[b],
            "vv8_in": vv8_full[b],
            "maskT": np.ascontiguousarray(maskT[b][:, q0:q0 + SQL]),
            "wq8T": wq8T,
            "wqr8T": wqr8T,
        })
    global LAST_RESULT
    res2 = run_bass_kernel_spmd(_get_l2(), l2_maps,
                                core_ids=list(range(N_CORES)), trace=TRACE)
    LAST_RESULT = res2
    o2 = res2.results

    # host finish: out = (raw[:, :H] + colsum(v_eff)) / (raw[:, H] + S)
    # v_eff comes straight from the chip's v8+vr8 tensors
    full = np.empty((B, S, H), dtype=np.float32)
    for bidx in range(B):
        vv = vv8_full[bidx].astype(np.float32)   # [NPR*P, 2, 2, 257]
        v_eff = vv[:, :, 0, :] + vv[:, :, 1, :]  # [NPR*P, 2, 257]
        colsum = v_eff[:, :, :H].astype(np.float64).sum(axis=(0, 1))
        for half in range(CORES_PER_BATCH):
            c = bidx * CORES_PER_BATCH + half
            raw = np.asarray(o2[c]["out"], np.float64)
            q0 = half * SQL
            num = raw[:, :H] + colsum[None, :]
            den = raw[:, H:H + 1] + float(S)
            full[bidx, q0:q0 + SQL] = (num / den).astype(np.float32)
    return full
